# revision 21
# baseline (speedup 1.0000x reference)
"""Bidirectional LSTM (B=64, T=256, D=512, U=500) on 8 Trainium2 NeuronCores.

Sharding: 2 directions x 4 batch-groups -> 16 samples per core, one direction
per core. Backward cores receive time-reversed x from the host, so the device
program is pure SPMD (identical on all 8 cores).

Per-core program:
  Phase 1 (GEMM): xz[t*16+b, 4U] = x @ Wk + b     (f32r matmuls, K=512, M=4096, N=2000)
  Phase 2 (recurrence), 256 steps:
      PSUM bank n is preloaded with xz_t gate-slice n via an identity
      copy-matmul (start=True), then the 4 recurrent matmuls accumulate
      h @ Wr on top (start=False) -> z lands fully-formed in PSUM and the
      per-gate vector adds disappear.
      Gate-group order on the PE is f,i,g,o so t2 = f*c starts early and
      the g-driven tail (tanh g -> c -> tanh c -> h) is as short as
      possible. The tail is chunked (2x250) to pipeline Act/DVE/PE.
      Keeping the PE queue dense (copy-matmuls + transposes fill the gap
      between steps) holds the tensor engine out of its half-rate HAM
      throttle state.
"""

import numpy as np

B, T, D, U = 64, 256, 512, 500
G4 = 4 * U            # 2000
NCORES = 8
BC = B // 4           # 16 samples per core
KCH, KQ = 4, 125      # U = 4 chunks of 125 (recurrent contraction)
DCH = 4               # D = 4 chunks of 128 (input contraction)
NSL = 500             # gate-slice / PSUM-bank width (<=512 fp32)
MT = (T * BC) // 128  # 32 M-tiles of 128 rows in the input GEMM
NSLOT = 4             # xz prefetch slots

_CACHE = {}


def _build_program(steps=T):
    import concourse.bass as bass
    import concourse.bacc as bacc
    import concourse.tile as tile
    import concourse.mybir as mybir
    from concourse.masks import make_identity

    dt = mybir.dt
    AF = mybir.ActivationFunctionType
    f32 = dt.float32
    f32r = dt.float32r
    bf16 = dt.bfloat16

    nc = bacc.Bacc("TRN2")

    xT = nc.dram_tensor("xT", [D, T * BC], f32r, kind="ExternalInput")  # (d, t*16+b)
    h0 = nc.dram_tensor("h0", [BC, U], f32, kind="ExternalInput")
    c0 = nc.dram_tensor("c0", [BC, U], f32, kind="ExternalInput")
    Wk = nc.dram_tensor("Wk", [D, G4], f32r, kind="ExternalInput")
    Wr = nc.dram_tensor("Wr", [U, G4], f32r, kind="ExternalInput")
    bv = nc.dram_tensor("b", [G4], f32, kind="ExternalInput")
    y = nc.dram_tensor("y", [T, BC, U], f32r, kind="ExternalOutput")
    xz = nc.dram_tensor("xzbuf", [T * BC, G4], f32r)

    with tile.TileContext(nc) as tc:
        with tc.tile_pool(name="persist", bufs=1) as persist:
            # Wr chunks stay resident for the whole kernel: chunk k = Wr[125k:125k+125, :]
            wr_sb = persist.tile([KQ, KCH, G4], f32r)
            for k in range(KCH):
                nc.gpsimd.dma_start(wr_sb[:, k, :], Wr[k * KQ:(k + 1) * KQ, :])
            ident_f = persist.tile([BC, BC], f32)
            make_identity(nc, ident_f)
            ident = persist.tile([BC, BC], f32r)
            nc.vector.tensor_copy(ident, ident_f)

            # ---------------- Phase 1: xz = x @ Wk + b ----------------
            with tc.tile_pool(name="gx", bufs=1) as gx, \
                 tc.tile_pool(name="gpsum", bufs=2, space="PSUM") as gps, \
                 tc.tile_pool(name="gout", bufs=3) as gout:
                xT_sb = gx.tile([128, DCH, T * BC], f32r)
                wk_sb = gx.tile([128, DCH, G4], f32r)
                for k in range(DCH):
                    nc.gpsimd.dma_start(xT_sb[:, k, :], xT[k * 128:(k + 1) * 128, :])
                    nc.gpsimd.dma_start(wk_sb[:, k, :], Wk[k * 128:(k + 1) * 128, :])
                b_bc = gx.tile([128, G4], f32)
                bva = bv[:]
                nc.gpsimd.dma_start(
                    b_bc, bass.AP(bva.tensor, bva.offset, [[0, 128], [1, G4]])
                )
                for m in range(MT):
                    ps = gps.tile([128, 4, 512], f32)
                    for n in range(4):
                        for k in range(DCH):
                            nc.tensor.matmul(
                                ps[:, n, 0:NSL],
                                lhsT=xT_sb[:, k, m * 128:(m + 1) * 128],
                                rhs=wk_sb[:, k, n * NSL:(n + 1) * NSL],
                                start=(k == 0),
                                stop=(k == DCH - 1),
                            )
                    so = gout.tile([128, G4], f32r)
                    for n in range(4):
                        nc.vector.tensor_add(
                            so[:, n * NSL:(n + 1) * NSL],
                            ps[:, n, 0:NSL],
                            b_bc[:, n * NSL:(n + 1) * NSL],
                        )
                    nc.sync.dma_start(xz[m * 128:(m + 1) * 128, :], so)

            # ---------------- Phase 2: recurrence ----------------
            with tc.tile_pool(name="state", bufs=2) as st, \
                 tc.tile_pool(name="gates", bufs=2) as gt, \
                 tc.tile_pool(name="rpsum", bufs=1, space="PSUM") as rps, \
                 tc.tile_pool(name="tpsum", bufs=2, space="PSUM") as tps:

                xzsl = persist.tile([BC, NSLOT, G4], f32r)

                h0t = st.tile([BC, U], f32, tag="h0t")
                c_sb = st.tile([BC, U], f32, tag="c")
                nc.sync.dma_start(h0t, h0[:, :])
                nc.sync.dma_start(c_sb, c0[:, :])
                h_sb = st.tile([BC, U], f32r, tag="h")
                nc.vector.tensor_copy(h_sb, h0t)

                # Prefetch xz for t = 0..2 on the Pool DMA queue.
                for t0 in range(min(3, steps)):
                    nc.gpsimd.dma_start(
                        xzsl[:, t0 % NSLOT, :], xz[t0 * BC:(t0 + 1) * BC, :]
                    )

                # gate-group order on the PE: f, i, g, o
                GORD = (1, 0, 2, 3)

                for t in range(steps):
                    slot = t % NSLOT
                    xzv = xzsl[:, slot, :]

                    if t + 3 < steps:
                        t3 = t + 3
                        nc.gpsimd.dma_start(
                            xzsl[:, t3 % NSLOT, :], xz[t3 * BC:(t3 + 1) * BC, :]
                        )

                    bank = [
                        rps.tile([BC, 512], f32, tag=f"ps{n}", name=f"ps{n}")
                        for n in range(4)
                    ]
                    tpt = tps.tile([KQ, KCH, BC], f32r, tag="tp")
                    hT = st.tile([KQ, KCH, BC], f32r, tag="hT")

                    def cp(n):
                        nc.tensor.matmul(
                            bank[n][:, 0:NSL],
                            lhsT=ident,
                            rhs=xzv[:, n * NSL:(n + 1) * NSL],
                            start=True,
                            stop=False,
                        )

                    def tr(j):
                        nc.tensor.transpose(
                            tpt[:, j, :], h_sb[:, j * KQ:(j + 1) * KQ], ident
                        )
                        nc.vector.tensor_copy(hT[:, j, :], tpt[:, j, :])

                    def mm(n, k):
                        nc.tensor.matmul(
                            bank[n][:, 0:NSL],
                            lhsT=hT[:, k, :],
                            rhs=wr_sb[:, k, n * NSL:(n + 1) * NSL],
                            start=False,
                            stop=(k == KCH - 1),
                            skip_group_check=True,
                        )

                    # PE: preload banks f,i by identity copy-matmul (fills the
                    # inter-step gap), transposes, then the 4 recurrent groups
                    # contiguous per bank so each gate's semaphore releases as
                    # soon as its group stops. Banks g,o are preloaded by the
                    # Vector engine right after the hT casts — their groups
                    # run late in the step, so the copies are off the critical
                    # path and shave 1000 rows off the HAM-throttled PE.
                    cp(GORD[0])
                    cp(GORD[1])
                    tr(0)
                    tr(1)
                    tr(2)
                    tr(3)
                    nc.vector.tensor_copy(bank[GORD[2]][:, 0:NSL],
                                          xzv[:, GORD[2] * NSL:(GORD[2] + 1) * NSL])
                    nc.vector.tensor_copy(bank[GORD[3]][:, 0:NSL],
                                          xzv[:, GORD[3] * NSL:(GORD[3] + 1) * NSL])
                    for n in GORD:
                        for k in range(KCH):
                            mm(n, k)

                    a = gt.tile([BC, 4, NSL], f32, tag="a")
                    t1 = st.tile([BC, U], f32, tag="t1")
                    t2 = st.tile([BC, U], f32, tag="t2")
                    c_new = st.tile([BC, U], f32, tag="c")
                    th = st.tile([BC, U], f32, tag="th")
                    h_new = st.tile([BC, U], f32r, tag="h")

                    # Act: f, i early; g / o / tanh(c) form the tail.
                    nc.scalar.activation(a[:, 1, :], bank[1][:, 0:NSL], AF.Sigmoid)
                    nc.scalar.activation(a[:, 0, :], bank[0][:, 0:NSL], AF.Sigmoid)
                    # DVE: t2 = f * c_old as soon as sig(f) lands.
                    nc.vector.tensor_mul(t2, a[:, 1, :], c_sb)

                    for lo, hi in ((0, 250), (250, 500)):
                        sl = slice(lo, hi)
                        nc.scalar.activation(a[:, 2, sl], bank[2][:, lo:hi], AF.Tanh)
                        nc.vector.tensor_mul(t1[:, sl], a[:, 0, sl], a[:, 2, sl])
                        nc.vector.tensor_add(c_new[:, sl], t1[:, sl], t2[:, sl])
                        nc.scalar.activation(a[:, 3, sl], bank[3][:, lo:hi], AF.Sigmoid)
                        nc.scalar.activation(th[:, sl], c_new[:, sl], AF.Tanh)
                        nc.vector.tensor_mul(h_new[:, sl], a[:, 3, sl], th[:, sl])

                    nc.sync.dma_start(y[t], h_new)
                    h_sb, c_sb = h_new, c_new
    nc.finalize()
    return nc


def _make_in_maps(x, h_f, c_f, h_b, c_b, Wk_f, Wr_f, b_f, Wk_b, Wr_b, b_b):
    x = np.ascontiguousarray(np.asarray(x, np.float32))
    in_maps = []
    for core in range(NCORES):
        d = core // 4           # 0 = forward, 1 = backward
        g = core % 4
        bs = slice(g * BC, (g + 1) * BC)
        xc = x[bs] if d == 0 else x[bs, ::-1]
        # xT[d, t*16+b] = xc[b, t, d]
        xTc = np.ascontiguousarray(xc.transpose(2, 1, 0).reshape(D, T * BC))
        in_maps.append({
            "xT": xTc,
            "h0": np.ascontiguousarray((h_f if d == 0 else h_b)[bs], np.float32),
            "c0": np.ascontiguousarray((c_f if d == 0 else c_b)[bs], np.float32),
            "Wk": np.ascontiguousarray(Wk_f if d == 0 else Wk_b, np.float32),
            "Wr": np.ascontiguousarray(Wr_f if d == 0 else Wr_b, np.float32),
            "b": np.ascontiguousarray(b_f if d == 0 else b_b, np.float32),
        })
    return in_maps


def kernel(x, h_f, c_f, h_b, c_b, Wk_f, Wr_f, b_f, Wk_b, Wr_b, b_b):
    from concourse.bass_utils import run_bass_kernel_spmd

    if "nc" not in _CACHE:
        _CACHE["nc"] = _build_program()
    nc = _CACHE["nc"]
    in_maps = _make_in_maps(x, h_f, c_f, h_b, c_b, Wk_f, Wr_f, b_f, Wk_b, Wr_b, b_b)

    import os
    trace = os.environ.get("BLSTM_TRACE") == "1"
    tmpdir = os.environ.get("BLSTM_TRACE_DIR") or None
    br = run_bass_kernel_spmd(nc, in_maps, list(range(NCORES)), trace=trace, tmpdir=tmpdir)
    _CACHE["exec_time_ns"] = br.exec_time_ns
    res = br.results

    out = np.empty((B, T, 2 * U), np.float32)
    for core in range(NCORES):
        d = core // 4
        g = core % 4
        yc = res[core]["y"]                    # [T, BC, U]
        yc = np.transpose(yc, (1, 0, 2))       # [BC, T, U]
        bs = slice(g * BC, (g + 1) * BC)
        if d == 0:
            out[bs, :, :U] = yc
        else:
            out[bs, :, U:] = yc[:, ::-1]
    return out


# revision 22
# speedup vs baseline: 1.0061x; 1.0061x over previous
"""Bidirectional LSTM (B=64, T=256, D=512, U=500) on 8 Trainium2 NeuronCores.

Sharding: 2 directions x 4 batch-groups -> 16 samples per core, one direction
per core. Backward cores receive time-reversed x from the host, so the device
program is pure SPMD (identical on all 8 cores).

Per-core program:
  Phase 1 (GEMM): xz[t*16+b, 4U] = x @ Wk + b     (f32r matmuls, K=512, M=4096, N=2000)
  Phase 2 (recurrence), 256 steps:
      PSUM bank n is preloaded with xz_t gate-slice n via an identity
      copy-matmul (start=True), then the 4 recurrent matmuls accumulate
      h @ Wr on top (start=False) -> z lands fully-formed in PSUM and the
      per-gate vector adds disappear.
      Gate-group order on the PE is f,i,g,o so t2 = f*c starts early and
      the g-driven tail (tanh g -> c -> tanh c -> h) is as short as
      possible. The tail is chunked (2x250) to pipeline Act/DVE/PE.
      Keeping the PE queue dense (copy-matmuls + transposes fill the gap
      between steps) holds the tensor engine out of its half-rate HAM
      throttle state.
"""

import numpy as np

B, T, D, U = 64, 256, 512, 500
G4 = 4 * U            # 2000
NCORES = 8
BC = B // 4           # 16 samples per core
KCH, KQ = 4, 125      # U = 4 chunks of 125 (recurrent contraction)
DCH = 4               # D = 4 chunks of 128 (input contraction)
NSL = 500             # gate-slice / PSUM-bank width (<=512 fp32)
MT = (T * BC) // 128  # 32 M-tiles of 128 rows in the input GEMM
NSLOT = 4             # xz prefetch slots

_CACHE = {}


def _build_program(steps=T):
    import concourse.bass as bass
    import concourse.bacc as bacc
    import concourse.tile as tile
    import concourse.mybir as mybir
    from concourse.masks import make_identity

    dt = mybir.dt
    AF = mybir.ActivationFunctionType
    f32 = dt.float32
    f32r = dt.float32r
    bf16 = dt.bfloat16

    nc = bacc.Bacc("TRN2")

    xT = nc.dram_tensor("xT", [D, T * BC], f32r, kind="ExternalInput")  # (d, t*16+b)
    h0 = nc.dram_tensor("h0", [BC, U], f32, kind="ExternalInput")
    c0 = nc.dram_tensor("c0", [BC, U], f32, kind="ExternalInput")
    Wk = nc.dram_tensor("Wk", [D, G4], f32r, kind="ExternalInput")
    Wr = nc.dram_tensor("Wr", [U, G4], f32r, kind="ExternalInput")
    bv = nc.dram_tensor("b", [G4], f32, kind="ExternalInput")
    y = nc.dram_tensor("y", [T, BC, U], f32r, kind="ExternalOutput")
    xz = nc.dram_tensor("xzbuf", [T * BC, G4], f32r)

    with tile.TileContext(nc) as tc:
        with tc.tile_pool(name="persist", bufs=1) as persist:
            # Wr chunks stay resident for the whole kernel: chunk k = Wr[125k:125k+125, :]
            wr_sb = persist.tile([KQ, KCH, G4], f32r)
            for k in range(KCH):
                nc.gpsimd.dma_start(wr_sb[:, k, :], Wr[k * KQ:(k + 1) * KQ, :])
            ident_f = persist.tile([BC, BC], f32)
            make_identity(nc, ident_f)
            ident = persist.tile([BC, BC], f32r)
            nc.vector.tensor_copy(ident, ident_f)

            # ---------------- Phase 1: xz = x @ Wk + b ----------------
            with tc.tile_pool(name="gx", bufs=1) as gx, \
                 tc.tile_pool(name="gpsum", bufs=2, space="PSUM") as gps, \
                 tc.tile_pool(name="gout", bufs=3) as gout:
                xT_sb = gx.tile([128, DCH, T * BC], f32r)
                wk_sb = gx.tile([128, DCH, G4], f32r)
                for k in range(DCH):
                    nc.gpsimd.dma_start(xT_sb[:, k, :], xT[k * 128:(k + 1) * 128, :])
                    nc.gpsimd.dma_start(wk_sb[:, k, :], Wk[k * 128:(k + 1) * 128, :])
                b_bc = gx.tile([128, G4], f32)
                bva = bv[:]
                nc.gpsimd.dma_start(
                    b_bc, bass.AP(bva.tensor, bva.offset, [[0, 128], [1, G4]])
                )
                for m in range(MT):
                    ps = gps.tile([128, 4, 512], f32)
                    for n in range(4):
                        for k in range(DCH):
                            nc.tensor.matmul(
                                ps[:, n, 0:NSL],
                                lhsT=xT_sb[:, k, m * 128:(m + 1) * 128],
                                rhs=wk_sb[:, k, n * NSL:(n + 1) * NSL],
                                start=(k == 0),
                                stop=(k == DCH - 1),
                            )
                    so = gout.tile([128, G4], f32r)
                    for n in range(4):
                        nc.vector.tensor_add(
                            so[:, n * NSL:(n + 1) * NSL],
                            ps[:, n, 0:NSL],
                            b_bc[:, n * NSL:(n + 1) * NSL],
                        )
                    nc.sync.dma_start(xz[m * 128:(m + 1) * 128, :], so)

            # ---------------- Phase 2: recurrence ----------------
            with tc.tile_pool(name="state", bufs=2) as st, \
                 tc.tile_pool(name="gates", bufs=2) as gt, \
                 tc.tile_pool(name="rpsum", bufs=1, space="PSUM") as rps, \
                 tc.tile_pool(name="tpsum", bufs=2, space="PSUM") as tps:

                xzsl = persist.tile([BC, NSLOT, G4], f32r)

                h0t = st.tile([BC, U], f32, tag="h0t")
                c_sb = st.tile([BC, U], f32, tag="c")
                nc.sync.dma_start(h0t, h0[:, :])
                nc.sync.dma_start(c_sb, c0[:, :])
                h_sb = st.tile([BC, U], f32r, tag="h")
                nc.vector.tensor_copy(h_sb, h0t)

                # Prefetch xz for t = 0..2 on the Pool DMA queue.
                for t0 in range(min(3, steps)):
                    nc.gpsimd.dma_start(
                        xzsl[:, t0 % NSLOT, :], xz[t0 * BC:(t0 + 1) * BC, :]
                    )

                # gate-group order on the PE: f, i, g, o
                GORD = (1, 0, 2, 3)

                for t in range(steps):
                    slot = t % NSLOT
                    xzv = xzsl[:, slot, :]

                    if t + 3 < steps:
                        t3 = t + 3
                        nc.gpsimd.dma_start(
                            xzsl[:, t3 % NSLOT, :], xz[t3 * BC:(t3 + 1) * BC, :]
                        )

                    bank = [
                        rps.tile([BC, 512], f32, tag=f"ps{n}", name=f"ps{n}")
                        for n in range(4)
                    ]
                    tpt = tps.tile([KQ, KCH, BC], f32r, tag="tp")
                    hT = st.tile([KQ, KCH, BC], f32r, tag="hT")

                    def cp(n):
                        nc.tensor.matmul(
                            bank[n][:, 0:NSL],
                            lhsT=ident,
                            rhs=xzv[:, n * NSL:(n + 1) * NSL],
                            start=True,
                            stop=False,
                        )

                    def tr(j):
                        nc.tensor.transpose(
                            tpt[:, j, :], h_sb[:, j * KQ:(j + 1) * KQ], ident
                        )
                        nc.vector.tensor_copy(hT[:, j, :], tpt[:, j, :])

                    def mm(n, k):
                        nc.tensor.matmul(
                            bank[n][:, 0:NSL],
                            lhsT=hT[:, k, :],
                            rhs=wr_sb[:, k, n * NSL:(n + 1) * NSL],
                            start=False,
                            stop=(k == KCH - 1),
                            skip_group_check=True,
                        )

                    # PE: all four bank preloads fill the inter-step gap, then
                    # the first gate group's k=0,1 matmuls slot in right after
                    # tr0/tr1 (they only need those two hT chunks), hiding the
                    # wait for the late h chunk behind real work. Groups stay
                    # contiguous enough that each gate's semaphore releases at
                    # its own stop.
                    cp(GORD[0])
                    cp(GORD[1])
                    cp(GORD[2])
                    cp(GORD[3])
                    tr(0)
                    tr(1)
                    mm(GORD[0], 0)
                    mm(GORD[0], 1)
                    tr(2)
                    tr(3)
                    mm(GORD[0], 2)
                    mm(GORD[0], 3)
                    for n in GORD[1:]:
                        for k in range(KCH):
                            mm(n, k)

                    a = gt.tile([BC, 4, NSL], f32, tag="a")
                    t1 = st.tile([BC, U], f32, tag="t1")
                    t2 = st.tile([BC, U], f32, tag="t2")
                    c_new = st.tile([BC, U], f32, tag="c")
                    th = st.tile([BC, U], f32, tag="th")
                    h_new = st.tile([BC, U], f32r, tag="h")

                    # Act: f, i early; g / o / tanh(c) form the tail.
                    nc.scalar.activation(a[:, 1, :], bank[1][:, 0:NSL], AF.Sigmoid)
                    nc.scalar.activation(a[:, 0, :], bank[0][:, 0:NSL], AF.Sigmoid)
                    # DVE: t2 = f * c_old as soon as sig(f) lands.
                    nc.vector.tensor_mul(t2, a[:, 1, :], c_sb)

                    for lo, hi in ((0, 250), (250, 500)):
                        sl = slice(lo, hi)
                        nc.scalar.activation(a[:, 2, sl], bank[2][:, lo:hi], AF.Tanh)
                        nc.vector.tensor_mul(t1[:, sl], a[:, 0, sl], a[:, 2, sl])
                        nc.vector.tensor_add(c_new[:, sl], t1[:, sl], t2[:, sl])
                        nc.scalar.activation(a[:, 3, sl], bank[3][:, lo:hi], AF.Sigmoid)
                        nc.scalar.activation(th[:, sl], c_new[:, sl], AF.Tanh)
                        nc.vector.tensor_mul(h_new[:, sl], a[:, 3, sl], th[:, sl])

                    nc.sync.dma_start(y[t], h_new)
                    h_sb, c_sb = h_new, c_new
    nc.finalize()
    return nc


def _make_in_maps(x, h_f, c_f, h_b, c_b, Wk_f, Wr_f, b_f, Wk_b, Wr_b, b_b):
    x = np.ascontiguousarray(np.asarray(x, np.float32))
    in_maps = []
    for core in range(NCORES):
        d = core // 4           # 0 = forward, 1 = backward
        g = core % 4
        bs = slice(g * BC, (g + 1) * BC)
        xc = x[bs] if d == 0 else x[bs, ::-1]
        # xT[d, t*16+b] = xc[b, t, d]
        xTc = np.ascontiguousarray(xc.transpose(2, 1, 0).reshape(D, T * BC))
        in_maps.append({
            "xT": xTc,
            "h0": np.ascontiguousarray((h_f if d == 0 else h_b)[bs], np.float32),
            "c0": np.ascontiguousarray((c_f if d == 0 else c_b)[bs], np.float32),
            "Wk": np.ascontiguousarray(Wk_f if d == 0 else Wk_b, np.float32),
            "Wr": np.ascontiguousarray(Wr_f if d == 0 else Wr_b, np.float32),
            "b": np.ascontiguousarray(b_f if d == 0 else b_b, np.float32),
        })
    return in_maps


def kernel(x, h_f, c_f, h_b, c_b, Wk_f, Wr_f, b_f, Wk_b, Wr_b, b_b):
    from concourse.bass_utils import run_bass_kernel_spmd

    if "nc" not in _CACHE:
        _CACHE["nc"] = _build_program()
    nc = _CACHE["nc"]
    in_maps = _make_in_maps(x, h_f, c_f, h_b, c_b, Wk_f, Wr_f, b_f, Wk_b, Wr_b, b_b)

    import os
    trace = os.environ.get("BLSTM_TRACE") == "1"
    tmpdir = os.environ.get("BLSTM_TRACE_DIR") or None
    br = run_bass_kernel_spmd(nc, in_maps, list(range(NCORES)), trace=trace, tmpdir=tmpdir)
    _CACHE["exec_time_ns"] = br.exec_time_ns
    res = br.results

    out = np.empty((B, T, 2 * U), np.float32)
    for core in range(NCORES):
        d = core // 4
        g = core % 4
        yc = res[core]["y"]                    # [T, BC, U]
        yc = np.transpose(yc, (1, 0, 2))       # [BC, T, U]
        bs = slice(g * BC, (g + 1) * BC)
        if d == 0:
            out[bs, :, :U] = yc
        else:
            out[bs, :, U:] = yc[:, ::-1]
    return out


# revision 25
# speedup vs baseline: 1.4104x; 1.4019x over previous
"""Bidirectional LSTM (B=64, T=256, D=512, U=500) on 8 Trainium2 NeuronCores.

Sharding: 2 directions x 4 batch-groups -> 16 samples per core, one direction
per core. Backward cores receive time-reversed x from the host, so the device
program is pure SPMD (identical on all 8 cores).

Per-core program:
  Phase 1 (GEMM): xz[t*16+b, 4U] = x @ Wk + b     (f32r matmuls, K=512, M=4096, N=2000)
  Phase 2 (recurrence), 256 steps:
      PSUM bank n is preloaded with xz_t gate-slice n via an identity
      copy-matmul (start=True), then the 4 recurrent matmuls accumulate
      h @ Wr on top (start=False) -> z lands fully-formed in PSUM and the
      per-gate vector adds disappear.
      Gate-group order on the PE is f,i,g,o so t2 = f*c starts early and
      the g-driven tail (tanh g -> c -> tanh c -> h) is as short as
      possible. The tail is chunked (2x250) to pipeline Act/DVE/PE.
      Keeping the PE queue dense (copy-matmuls + transposes fill the gap
      between steps) holds the tensor engine out of its half-rate HAM
      throttle state.
"""

import numpy as np

B, T, D, U = 64, 256, 512, 500
G4 = 4 * U            # 2000
NCORES = 8
BC = B // 4           # 16 samples per core
KCH, KQ = 4, 125      # U = 4 chunks of 125 (recurrent contraction)
DCH = 4               # D = 4 chunks of 128 (input contraction)
NSL = 500             # gate-slice / PSUM-bank width (<=512 fp32)
MT = (T * BC) // 128  # 32 M-tiles of 128 rows in the input GEMM
NSLOT = 4             # xz prefetch slots

_CACHE = {}


def _build_program(steps=T):
    import concourse.bass as bass
    import concourse.bacc as bacc
    import concourse.tile as tile
    import concourse.mybir as mybir
    from concourse.masks import make_identity

    dt = mybir.dt
    AF = mybir.ActivationFunctionType
    f32 = dt.float32
    f32r = dt.float32r
    bf16 = dt.bfloat16

    nc = bacc.Bacc("TRN2")

    xT = nc.dram_tensor("xT", [D, T * BC], f32r, kind="ExternalInput")  # (d, t*16+b)
    h0 = nc.dram_tensor("h0", [BC, U], f32, kind="ExternalInput")
    c0 = nc.dram_tensor("c0", [BC, U], f32, kind="ExternalInput")
    Wk = nc.dram_tensor("Wk", [D, G4], f32r, kind="ExternalInput")
    Wr = nc.dram_tensor("Wr", [U, G4], f32r, kind="ExternalInput")
    bv = nc.dram_tensor("b", [G4], f32, kind="ExternalInput")
    y = nc.dram_tensor("y", [T, BC, U], f32r, kind="ExternalOutput")
    xz = nc.dram_tensor("xzbuf", [T * BC, G4], f32r)

    with tile.TileContext(nc) as tc:
        with tc.tile_pool(name="persist", bufs=1) as persist:
            # Wr chunks stay resident for the whole kernel: chunk k = Wr[125k:125k+125, :]
            wr_sb = persist.tile([KQ, KCH, G4], f32r)
            for k in range(KCH):
                nc.gpsimd.dma_start(wr_sb[:, k, :], Wr[k * KQ:(k + 1) * KQ, :])
            ident_f = persist.tile([BC, BC], f32)
            make_identity(nc, ident_f)
            ident = persist.tile([BC, BC], f32r)
            nc.vector.tensor_copy(ident, ident_f)

            # ---------------- Phase 1: xz = x @ Wk + b ----------------
            with tc.tile_pool(name="gx", bufs=1) as gx, \
                 tc.tile_pool(name="gpsum", bufs=2, space="PSUM") as gps, \
                 tc.tile_pool(name="gout", bufs=3) as gout:
                xT_sb = gx.tile([128, DCH, T * BC], f32r)
                wk_sb = gx.tile([128, DCH, G4], f32r)
                for k in range(DCH):
                    nc.gpsimd.dma_start(xT_sb[:, k, :], xT[k * 128:(k + 1) * 128, :])
                    nc.gpsimd.dma_start(wk_sb[:, k, :], Wk[k * 128:(k + 1) * 128, :])
                b_bc = gx.tile([128, G4], f32)
                bva = bv[:]
                nc.gpsimd.dma_start(
                    b_bc, bass.AP(bva.tensor, bva.offset, [[0, 128], [1, G4]])
                )
                for m in range(MT):
                    ps = gps.tile([128, 4, 512], f32)
                    for n in range(4):
                        for k in range(DCH):
                            nc.tensor.matmul(
                                ps[:, n, 0:NSL],
                                lhsT=xT_sb[:, k, m * 128:(m + 1) * 128],
                                rhs=wk_sb[:, k, n * NSL:(n + 1) * NSL],
                                start=(k == 0),
                                stop=(k == DCH - 1),
                            )
                    so = gout.tile([128, G4], f32r)
                    for n in range(4):
                        nc.vector.tensor_add(
                            so[:, n * NSL:(n + 1) * NSL],
                            ps[:, n, 0:NSL],
                            b_bc[:, n * NSL:(n + 1) * NSL],
                        )
                    nc.sync.dma_start(xz[m * 128:(m + 1) * 128, :], so)

            # ---------------- Phase 2: recurrence ----------------
            with tc.tile_pool(name="state", bufs=2) as st, \
                 tc.tile_pool(name="gates", bufs=2) as gt, \
                 tc.tile_pool(name="rpsum", bufs=1, space="PSUM") as rps, \
                 tc.tile_pool(name="tpsum", bufs=2, space="PSUM") as tps:

                xzsl = persist.tile([BC, NSLOT, G4], f32r)

                h0t = st.tile([BC, U], f32, tag="h0t")
                c_sb = st.tile([BC, U], f32, tag="c")
                nc.sync.dma_start(h0t, h0[:, :])
                nc.sync.dma_start(c_sb, c0[:, :])
                h_sb = st.tile([BC, U], f32r, tag="h")
                nc.vector.tensor_copy(h_sb, h0t)

                # Prefetch xz for t = 0..2 on the Pool DMA queue.
                for t0 in range(min(3, steps)):
                    nc.gpsimd.dma_start(
                        xzsl[:, t0 % NSLOT, :], xz[t0 * BC:(t0 + 1) * BC, :]
                    )

                # gate-group order on the PE: f, i, g, o
                GORD = (1, 0, 2, 3)

                for t in range(steps):
                    slot = t % NSLOT
                    xzv = xzsl[:, slot, :]

                    if t + 3 < steps:
                        t3 = t + 3
                        nc.gpsimd.dma_start(
                            xzsl[:, t3 % NSLOT, :], xz[t3 * BC:(t3 + 1) * BC, :]
                        )

                    bank = [
                        rps.tile([BC, 512], f32, tag=f"ps{n}", name=f"ps{n}")
                        for n in range(4)
                    ]
                    tpt = tps.tile([KQ, KCH, BC], f32r, tag="tp")
                    hT = st.tile([KQ, KCH, BC], f32r, tag="hT")

                    def cp(n):
                        nc.tensor.matmul(
                            bank[n][:, 0:NSL],
                            lhsT=ident,
                            rhs=xzv[:, n * NSL:(n + 1) * NSL],
                            start=True,
                            stop=False,
                        )

                    def tr(j):
                        nc.tensor.transpose(
                            tpt[:, j, :], h_sb[:, j * KQ:(j + 1) * KQ], ident
                        )
                        nc.vector.tensor_copy(hT[:, j, :], tpt[:, j, :])

                    def mm(n, k):
                        nc.tensor.matmul(
                            bank[n][:, 0:NSL],
                            lhsT=hT[:, k, :],
                            rhs=wr_sb[:, k, n * NSL:(n + 1) * NSL],
                            start=False,
                            stop=(k == KCH - 1),
                            skip_group_check=True,
                        )

                    # PE: all four bank preloads + keep-warm fill the
                    # inter-step gap, transposes as h chunks land, then the 4
                    # recurrent groups contiguous per bank so each gate's
                    # semaphore releases at its own stop.
                    cp(GORD[0])
                    cp(GORD[1])
                    cp(GORD[2])
                    cp(GORD[3])
                    tr(0)
                    tr(1)
                    tr(2)
                    tr(3)
                    for n in GORD:
                        for k in range(KCH):
                            mm(n, k)

                    a = gt.tile([BC, 4, NSL], f32, tag="a")
                    t1 = st.tile([BC, U], f32, tag="t1")
                    t2 = st.tile([BC, U], f32, tag="t2")
                    c_new = st.tile([BC, U], f32, tag="c")
                    th = st.tile([BC, U], f32, tag="th")
                    h_new = st.tile([BC, U], f32r, tag="h")

                    # Act: f, i early; g / o / tanh(c) form the tail.
                    nc.scalar.activation(a[:, 1, :], bank[1][:, 0:NSL], AF.Sigmoid)
                    nc.scalar.activation(a[:, 0, :], bank[0][:, 0:NSL], AF.Sigmoid)
                    # DVE: t2 = f * c_old as soon as sig(f) lands.
                    nc.vector.tensor_mul(t2, a[:, 1, :], c_sb)

                    # g / c / tanh(c) in 250-wide chunks; the o-gate and h in
                    # 125-wide chunks so each h quarter releases its transpose
                    # (next step's matmul input) as early as possible.
                    for lo, hi in ((0, 250), (250, 500)):
                        sl = slice(lo, hi)
                        nc.scalar.activation(a[:, 2, sl], bank[2][:, lo:hi], AF.Tanh)
                        nc.vector.tensor_mul(t1[:, sl], a[:, 0, sl], a[:, 2, sl])
                        nc.vector.tensor_add(c_new[:, sl], t1[:, sl], t2[:, sl])
                        nc.scalar.activation(th[:, sl], c_new[:, sl], AF.Tanh)
                    for q in range(KCH):
                        sl = slice(q * KQ, (q + 1) * KQ)
                        nc.scalar.activation(a[:, 3, sl], bank[3][:, q * KQ:(q + 1) * KQ], AF.Sigmoid)
                        nc.vector.tensor_mul(h_new[:, sl], a[:, 3, sl], th[:, sl])

                    nc.sync.dma_start(y[t], h_new)
                    h_sb, c_sb = h_new, c_new
    nc.finalize()
    return nc


def _make_in_maps(x, h_f, c_f, h_b, c_b, Wk_f, Wr_f, b_f, Wk_b, Wr_b, b_b):
    x = np.ascontiguousarray(np.asarray(x, np.float32))
    in_maps = []
    for core in range(NCORES):
        d = core // 4           # 0 = forward, 1 = backward
        g = core % 4
        bs = slice(g * BC, (g + 1) * BC)
        xc = x[bs] if d == 0 else x[bs, ::-1]
        # xT[d, t*16+b] = xc[b, t, d]
        xTc = np.ascontiguousarray(xc.transpose(2, 1, 0).reshape(D, T * BC))
        in_maps.append({
            "xT": xTc,
            "h0": np.ascontiguousarray((h_f if d == 0 else h_b)[bs], np.float32),
            "c0": np.ascontiguousarray((c_f if d == 0 else c_b)[bs], np.float32),
            "Wk": np.ascontiguousarray(Wk_f if d == 0 else Wk_b, np.float32),
            "Wr": np.ascontiguousarray(Wr_f if d == 0 else Wr_b, np.float32),
            "b": np.ascontiguousarray(b_f if d == 0 else b_b, np.float32),
        })
    return in_maps


def kernel(x, h_f, c_f, h_b, c_b, Wk_f, Wr_f, b_f, Wk_b, Wr_b, b_b):
    from concourse.bass_utils import run_bass_kernel_spmd

    if "nc" not in _CACHE:
        _CACHE["nc"] = _build_program()
    nc = _CACHE["nc"]
    in_maps = _make_in_maps(x, h_f, c_f, h_b, c_b, Wk_f, Wr_f, b_f, Wk_b, Wr_b, b_b)

    import os
    trace = os.environ.get("BLSTM_TRACE") == "1"
    tmpdir = os.environ.get("BLSTM_TRACE_DIR") or None
    br = run_bass_kernel_spmd(nc, in_maps, list(range(NCORES)), trace=trace, tmpdir=tmpdir)
    _CACHE["exec_time_ns"] = br.exec_time_ns
    res = br.results

    out = np.empty((B, T, 2 * U), np.float32)
    for core in range(NCORES):
        d = core // 4
        g = core % 4
        yc = res[core]["y"]                    # [T, BC, U]
        yc = np.transpose(yc, (1, 0, 2))       # [BC, T, U]
        bs = slice(g * BC, (g + 1) * BC)
        if d == 0:
            out[bs, :, :U] = yc
        else:
            out[bs, :, U:] = yc[:, ::-1]
    return out
